# revision 15
# baseline (speedup 1.0000x reference)
"""RWKV attention-prep kernel for 8 Trainium2 NeuronCores.

Computes, for hidden [B=4, T=4096, H=2048]:
  shifted = concat(attn_x, hidden[:, :-1])          (token shift)
  xxx     = x + (shifted - x) * time_maa_x
  l       = tanh(xxx @ time_maa_w1)                 (5 branches x rank 32)
  mf      = l_f @ time_maa_w2[f]
  in_f    = shifted + time_maa_f + mf               (note x + xx == shifted)
  receptance/key/value = in_f @ W_f
  gate    = silu(in_g @ gate_w)
  td      = time_decay + tanh(in_w @ dw1) @ dw2
  attn_x_new = hidden[:, -1]

Distribution: data-parallel over the 16384 tokens -> 2048 tokens/core
(core c handles batch c//2, half c%2), with a 1-token halo for the shift.

Algebraic restructuring (exact up to fp reassociation):
  in_f @ W_f = shifted @ W_f + l_f @ (w2_f @ W_f) + (maa_f @ W_f)
so the device never materializes mf/in_f; it streams shifted^T against W_f
with a rank-32 PSUM correction (V_f = w2_f @ W_f, folded on host) and a
per-column constant c_f = maa_f @ W_f added during PSUM eviction.
Similarly tanh(xxx @ w1) = tanh(x @ (w1*(1-maa_x)) + shifted @ (w1*maa_x)).

Matmuls run in float32r (full PE rate, ~1.5e-4 rel err vs 4x-slower fp32).
"""

import sys

sys.path.insert(0, "/opt/trn_rl_repo")

import numpy as np

import concourse.bacc as bacc
import concourse.mybir as mybir
from concourse.bass_utils import run_bass_kernel_spmd
from concourse.tile import TileContext

F32 = mybir.dt.float32
F32R = mybir.dt.float32r
AF = mybir.ActivationFunctionType
ALU = mybir.AluOpType

H = 2048
KT = H // 128          # 16 k-tiles
TOKC = 2048            # tokens per core
HALVES = 2
TOKH = TOKC // HALVES  # 1024
MT = TOKH // 128       # 8 m-tiles per half
NT = H // 512          # 4 n-tiles
TCH = TOKH // 512      # 2 token chunks (N=512) per half for lora/decay
NCORES = 8

# f-loop order; base = partition row of this branch's l rows in lbuf and of
# V_f rows in vbuf.  g lives in lbuf1 (base 0), others in lbuf0.
FCFG = [
    ("k", 32),
    ("v", 64),
    ("r", 96),
    ("g", 0),
]

_prog_cache = {}


class _Runner:
    """Cached jitted SPMD executor (mirrors bass2jax.run_bass_via_pjrt but
    reusable across calls, so repeat executions don't re-trace/re-compile)."""

    def __init__(self, nc, n_cores):
        import jax
        from jax.experimental.shard_map import shard_map
        from jax.sharding import Mesh, PartitionSpec

        from concourse import bass2jax, mybir as mb

        bass2jax.install_neuronx_cc_hook()
        self.jax = jax
        self.n_cores = n_cores

        part_name = nc.partition_id_tensor.name if nc.partition_id_tensor else None
        in_names, out_names, out_avals, zero_shapes = [], [], [], []
        for alloc in nc.m.functions[0].allocations:
            if not isinstance(alloc, mb.MemoryLocationSet):
                continue
            name = alloc.memorylocations[0].name
            if alloc.kind == "ExternalInput":
                if name != part_name:
                    in_names.append(name)
            elif alloc.kind == "ExternalOutput":
                out_names.append(name)
                shape = tuple(alloc.tensor_shape)
                dtype = mb.dt.np(alloc.dtype)
                out_avals.append(jax.core.ShapedArray(shape, dtype))
                zero_shapes.append((shape, dtype))
        self.in_names = in_names
        self.out_names = out_names
        self.out_avals = out_avals
        self.zero_shapes = zero_shapes
        n_params = len(in_names)

        bind_names = in_names + out_names
        if part_name is not None:
            bind_names = bind_names + [part_name]

        def _body(*args):
            operands = list(args)
            if part_name is not None:
                operands.append(bass2jax.partition_id_tensor())
            outs = bass2jax._bass_exec_p.bind(
                *operands,
                out_avals=tuple(out_avals),
                in_names=tuple(bind_names),
                out_names=tuple(out_names),
                lowering_input_output_aliases=(),
                sim_require_finite=True,
                sim_require_nnan=True,
                nc=nc,
            )
            return tuple(outs)

        devices = jax.devices()[:n_cores]
        mesh = Mesh(np.asarray(devices), ("core",))
        self.mesh = mesh
        self.PartitionSpec = PartitionSpec
        n_outs = len(out_names)
        self._fn = jax.jit(
            shard_map(
                _body,
                mesh=mesh,
                in_specs=(PartitionSpec("core"),) * (n_params + n_outs),
                out_specs=(PartitionSpec("core"),) * n_outs,
                check_rep=False,
            ),
            keep_unused=True,
        )

    def prep(self, in_maps):
        concat = [
            np.concatenate([np.asarray(m[name]) for m in in_maps], axis=0)
            for name in self.in_names
        ]
        zeros = [
            np.zeros((self.n_cores * s[0], *s[1:]), d) for s, d in self.zero_shapes
        ]
        return concat + zeros

    def run(self, args):
        out_arrs = self._fn(*args)
        self.jax.block_until_ready(out_arrs)
        return [
            {
                name: np.asarray(out_arrs[i]).reshape(
                    self.n_cores, *self.out_avals[i].shape
                )[c]
                for i, name in enumerate(self.out_names)
            }
            for c in range(self.n_cores)
        ]

    def bench(self, args, iters=10):
        import time

        from jax.sharding import NamedSharding

        # pre-shard inputs across the 8 cores so per-call timing excludes
        # host->device transfer and cross-core resharding
        sh = NamedSharding(self.mesh, self.PartitionSpec("core"))
        dev_args = [self.jax.device_put(a, sh) for a in args]
        out = self._fn(*dev_args)
        self.jax.block_until_ready(out)
        times = []
        for _ in range(iters):
            t0 = time.perf_counter()
            out = self._fn(*dev_args)
            self.jax.block_until_ready(out)
            times.append(time.perf_counter() - t0)
        return min(times), sorted(times)[len(times) // 2]


def _build_program():
    nc = bacc.Bacc("TRN2", target_bir_lowering=False, debug=False)

    xh = nc.dram_tensor("xh", (H, TOKC + 1), F32R, kind="ExternalInput")
    wf = {
        name: nc.dram_tensor(f"w_{name}", (H, H), F32R, kind="ExternalInput")
        for name, _ in FCFG
    }
    w1ab_d = nc.dram_tensor("w1ab", (H, 320), F32R, kind="ExternalInput")
    vb_d = nc.dram_tensor("vb", (128, H), F32R, kind="ExternalInput")
    cb_d = nc.dram_tensor("cb", (4, 128, H), F32, kind="ExternalInput")
    dw1_d = nc.dram_tensor("dw1", (H, 65), F32R, kind="ExternalInput")
    vw1_d = nc.dram_tensor("vw1", (32, 65), F32R, kind="ExternalInput")
    cw1_d = nc.dram_tensor("cw1", (65, 1), F32, kind="ExternalInput")
    dw2_d = nc.dram_tensor("dw2aug", (65, H), F32R, kind="ExternalInput")

    outs = {
        name: nc.dram_tensor(f"out_{name}", (TOKC, H), F32, kind="ExternalOutput")
        for name, _ in FCFG
    }
    out_td = nc.dram_tensor("out_td", (TOKC, H), F32, kind="ExternalOutput")

    with TileContext(nc) as tc:
        with (
            tc.tile_pool(name="pact", bufs=16) as pact,
            tc.tile_pool(name="pw", bufs=32) as pw,
            tc.tile_pool(name="pconst", bufs=1) as pconst,
            tc.tile_pool(name="plb", bufs=1) as plb,
            tc.tile_pool(name="pcb", bufs=2) as pcb,
            tc.tile_pool(name="pst", bufs=6) as pst,
            tc.tile_pool(name="pps", bufs=1, space="PSUM") as pps,
        ):
            # constants resident for the whole kernel
            vbuf = pconst.tile([128, H], F32R)
            nc.sync.dma_start(vbuf[:], vb_d[:])
            dw1t = [pconst.tile([128, 65], F32R, name=f"dw1t{k}") for k in range(KT)]
            for k in range(KT):
                nc.sync.dma_start(dw1t[k][:], dw1_d[k * 128 : (k + 1) * 128, :])
            vw1t = pconst.tile([32, 65], F32R)
            nc.sync.dma_start(vw1t[:], vw1_d[:])
            cw1t = pconst.tile([65, 1], F32)
            nc.sync.dma_start(cw1t[:], cw1_d[:])
            dw2t = pconst.tile([65, H], F32R)
            nc.sync.dma_start(dw2t[:], dw2_d[:])

            for half in range(HALVES):
                t0 = half * TOKH

                act = []
                for k in range(KT):
                    a = pact.tile([128, TOKH + 1], F32R, name=f"act{k}", tag="a")
                    nc.sync.dma_start(a[:], xh[k * 128 : (k + 1) * 128, t0 : t0 + TOKH + 1])
                    act.append(a)

                w1t = []
                for k in range(KT):
                    t = pw.tile([128, 512], F32R, name=f"w1t{k}", tag="w")
                    nc.sync.dma_start(t[:, :320], w1ab_d[k * 128 : (k + 1) * 128, :])
                    w1t.append(t)

                lbuf0 = plb.tile([128, TOKH], F32R, name="lbuf0", tag="lb0")
                lbuf1 = plb.tile([32, TOKH], F32R, name="lbuf1", tag="lb1")
                l2buf = plb.tile([65, TOKH], F32R, name="l2buf", tag="lb2")

                # ---- lora-1: l = tanh(x @ w1a + shifted @ w1b), transposed ----
                for tc_i in range(TCH):
                    tt = tc_i * 512
                    pl0 = pps.tile([128, 512], F32, name="po", tag="po", bufs=6)
                    for k in range(KT):
                        nc.tensor.matmul(
                            pl0[:], w1t[k][:, 0:128], act[k][:, 1 + tt : 513 + tt],
                            start=(k == 0), stop=False,
                        )
                        nc.tensor.matmul(
                            pl0[:], w1t[k][:, 160:288], act[k][:, tt : 512 + tt],
                            start=False, stop=(k == KT - 1),
                        )
                    nc.scalar.activation(lbuf0[:, tt : tt + 512], pl0[:], AF.Tanh)

                    pl1 = pps.tile([32, 512], F32, name="pl1", tag="pl1", bufs=1)
                    for k in range(KT):
                        nc.tensor.matmul(
                            pl1[:], w1t[k][:, 128:160], act[k][:, 1 + tt : 513 + tt],
                            start=(k == 0), stop=False,
                        )
                        nc.tensor.matmul(
                            pl1[:], w1t[k][:, 288:320], act[k][:, tt : 512 + tt],
                            start=False, stop=(k == KT - 1),
                        )
                    nc.scalar.activation(lbuf1[:, tt : tt + 512], pl1[:], AF.Tanh)

                # ---- decay mid: l2 = tanh(shifted @ dw1 + l_w @ vw1 + cw1), T ----
                for tc_i in range(TCH):
                    tt = tc_i * 512
                    pd = pps.tile([65, 512], F32, name="pd", tag="pd", bufs=1)
                    for k in range(KT):
                        nc.tensor.matmul(
                            pd[:], dw1t[k][:], act[k][:, tt : 512 + tt],
                            start=(k == 0), stop=False,
                        )
                    nc.tensor.matmul(
                        pd[:], vw1t[:], lbuf0[0:32, tt : tt + 512],
                        start=False, stop=True,
                    )
                    nc.scalar.activation(
                        l2buf[:, tt : tt + 512], pd[:], AF.Tanh, bias=cw1t[:, 0:1]
                    )

                # ---- decay final: td = l2aug @ dw2aug ----
                for n in range(NT):
                    for m in range(MT):
                        ptd = pps.tile([128, 512], F32, name="po", tag="po", bufs=6)
                        nc.tensor.matmul(
                            ptd[:],
                            l2buf[:, m * 128 : (m + 1) * 128],
                            dw2t[:, n * 512 : (n + 1) * 512],
                            start=True, stop=True,
                        )
                        st = pst.tile([128, 512], F32, name="st", tag="st")
                        nc.vector.tensor_copy(st[:], ptd[:])
                        nc.sync.dma_start(
                            out_td[t0 + m * 128 : t0 + (m + 1) * 128,
                                   n * 512 : (n + 1) * 512],
                            st[:],
                        )

                # ---- big branches ----
                for fi, (fname, base) in enumerate(FCFG):
                    lb = lbuf1 if fname == "g" else lbuf0
                    cbt = pcb.tile([128, H], F32, name="cbt", tag="cb")
                    nc.sync.dma_start(cbt[:], cb_d[fi])
                    for n in range(NT):
                        wblk = []
                        for k in range(KT):
                            t = pw.tile([128, 512], F32R, name=f"wb{k}", tag="w")
                            nc.sync.dma_start(
                                t[:],
                                wf[fname][k * 128 : (k + 1) * 128,
                                          n * 512 : (n + 1) * 512],
                            )
                            wblk.append(t)
                        for m in range(MT):
                            po = pps.tile([128, 512], F32, name="po", tag="po", bufs=6)
                            for k in range(KT):
                                nc.tensor.matmul(
                                    po[:],
                                    act[k][:, m * 128 : 128 + m * 128],
                                    wblk[k][:],
                                    start=(k == 0), stop=False,
                                )
                            nc.tensor.matmul(
                                po[:],
                                lb[base : base + 32, m * 128 : (m + 1) * 128],
                                vbuf[base : base + 32, n * 512 : (n + 1) * 512],
                                start=False, stop=True,
                                tile_position=(base, 0) if base == 96 else None,
                            )
                            st = pst.tile([128, 512], F32, name="st", tag="st")
                            nc.vector.tensor_tensor(
                                st[:], po[:], cbt[:, n * 512 : (n + 1) * 512], ALU.add
                            )
                            if fname == "g":
                                nc.scalar.activation(st[:], st[:], AF.Silu)
                            nc.sync.dma_start(
                                outs[fname][t0 + m * 128 : t0 + (m + 1) * 128,
                                            n * 512 : (n + 1) * 512],
                                st[:],
                            )

    nc.finalize()
    return nc


def _host_prep(inputs):
    """Fold weights and build per-core input maps."""
    hidden = np.asarray(inputs["hidden"], np.float32)
    attn_x = np.asarray(inputs["attn_x"], np.float32)
    B, T, h = hidden.shape
    assert (B * T, h) == (NCORES * TOKC, H)

    maa_x = np.asarray(inputs["time_maa_x"], np.float64).reshape(H)
    w1 = np.asarray(inputs["time_maa_w1"], np.float64)          # [H, 160]
    w2 = np.asarray(inputs["time_maa_w2"], np.float64)          # [5, 32, H]
    maa = {
        "w": np.asarray(inputs["time_maa_w"], np.float64).reshape(H),
        "k": np.asarray(inputs["time_maa_k"], np.float64).reshape(H),
        "v": np.asarray(inputs["time_maa_v"], np.float64).reshape(H),
        "r": np.asarray(inputs["time_maa_r"], np.float64).reshape(H),
        "g": np.asarray(inputs["time_maa_g"], np.float64).reshape(H),
    }
    W = {
        "r": np.asarray(inputs["receptance_w"], np.float64),
        "k": np.asarray(inputs["key_w"], np.float64),
        "v": np.asarray(inputs["value_w"], np.float64),
        "g": np.asarray(inputs["gate_w"], np.float64),
    }
    branch = {"w": 0, "k": 1, "v": 2, "r": 3, "g": 4}
    dw1 = np.asarray(inputs["time_decay_w1"], np.float64)       # [H, 64]
    dw2 = np.asarray(inputs["time_decay_w2"], np.float64)       # [64, H]
    tdec = np.asarray(inputs["time_decay"], np.float64).reshape(H)

    # lora-1 split weights: xxx @ w1 == x @ w1a + shifted @ w1b
    w1ab = np.empty((H, 320), np.float32)
    w1ab[:, 0:160] = (w1 * (1.0 - maa_x)[:, None]).astype(np.float32)
    w1ab[:, 160:320] = (w1 * maa_x[:, None]).astype(np.float32)

    # folded rank-32 corrections V_f = w2_f @ W_f, packed by partition base
    vb = np.empty((128, H), np.float32)
    cb = np.empty((4, 128, H), np.float32)
    for fi, (fname, base) in enumerate(FCFG):
        vb[base : base + 32] = (w2[branch[fname]] @ W[fname]).astype(np.float32)
        cb[fi] = np.broadcast_to(
            (maa[fname] @ W[fname]).astype(np.float32), (128, H)
        )

    # decay weights padded with a 65th zero column; tanh(0 + bias 20) == 1.0
    # exactly in fp32, which supplies the ones-row of l2aug that carries the
    # "+ time_decay" term through the final matmul.
    dw1p = np.zeros((H, 65), np.float32)
    dw1p[:, 0:64] = dw1.astype(np.float32)
    vw1 = np.zeros((32, 65), np.float32)
    vw1[:, 0:64] = (w2[0] @ dw1).astype(np.float32)
    cw1 = np.zeros((65, 1), np.float32)
    cw1[0:64, 0] = (maa["w"] @ dw1).astype(np.float32)
    cw1[64, 0] = 20.0
    dw2aug = np.empty((65, H), np.float32)
    dw2aug[0:64] = dw2.astype(np.float32)
    dw2aug[64] = tdec.astype(np.float32)

    shared = {
        "w1ab": w1ab, "vb": vb, "cb": cb, "dw1": dw1p,
        "vw1": vw1, "cw1": cw1, "dw2aug": dw2aug,
        "w_r": np.asarray(inputs["receptance_w"], np.float32),
        "w_k": np.asarray(inputs["key_w"], np.float32),
        "w_v": np.asarray(inputs["value_w"], np.float32),
        "w_g": np.asarray(inputs["gate_w"], np.float32),
    }

    in_maps = []
    for c in range(NCORES):
        b, seg = divmod(c, HALVES)
        toks = hidden[b, seg * TOKC : (seg + 1) * TOKC]         # [TOKC, H]
        prev = attn_x[b] if seg == 0 else hidden[b, seg * TOKC - 1]
        xh = np.empty((TOKC + 1, H), np.float32)
        xh[0] = prev
        xh[1:] = toks
        m = dict(shared)
        m["xh"] = np.ascontiguousarray(xh.T)                    # [H, TOKC+1]
        in_maps.append(m)
    return in_maps


def _get_runner():
    if "runner" not in _prog_cache:
        nc = _build_program()
        _prog_cache["runner"] = _Runner(nc, NCORES)
    return _prog_cache["runner"]


def _assemble(inputs, results):
    hidden = np.asarray(inputs["hidden"], np.float32)
    B, T, h = hidden.shape
    full = {
        name: np.empty((B, T, H), np.float32) for name in ("r", "k", "v", "g", "td")
    }
    for c in range(NCORES):
        b, seg = divmod(c, HALVES)
        sl = slice(seg * TOKC, (seg + 1) * TOKC)
        r = results[c]
        for name, _ in FCFG:
            full[name][b, sl] = r[f"out_{name}"]
        full["td"][b, sl] = r["out_td"]
    attn_x_new = np.ascontiguousarray(hidden[:, -1])
    return (full["r"], full["k"], full["v"], full["g"], full["td"], attn_x_new)


def kernel(**inputs):
    runner = _get_runner()
    args = runner.prep(_host_prep(inputs))
    results = runner.run(args)
    return _assemble(inputs, results)
